# revision 44
# baseline (speedup 1.0000x reference)
"""Trainium2 Bass kernel for a GPT-2 style transformer block (fp8 DoubleRow).

Sharding across 8 NeuronCores: cores 0-3 handle batch 0, cores 4-7 batch 1.
Within each 4-core group: tensor-parallel attention (3 heads/core, each core
computes LN1 + its QKV shard for the full 2048 tokens -> no communication),
row-sharded c_proj partials, then TWO half-token ReduceScatters (core r owns
tokens [256r:256r+256] and [1024+256r:+256]), after which each core owns 512
tokens and runs the MLP token-parallel.

v2 changes vs baseline:
 - All non-attention matmuls (QKV, c_proj, fc, mproj) run as fp8e4m3
   DoubleRow matmuls (256-wide contraction per instruction): weights are
   host-quantized fp8 at power-of-2 scales; activations are cast to fp8 at
   the PSUM->SBUF copies that already existed. Scales unfold for free via
   the exp/gelu activation `scale` and the existing output tensor_scalar.
 - x arrives bf16 (halves the input DMA; residual path keeps f32 xs).
 - Softmax: denominator divide is fused into the yT normalize (no 512-wide
   reciprocal), causal mask mult shrunk to the 128x128 diagonal block.
 - Engine rebalance: LN applies on DVE (2x bf16 mode), transpose copies on
   ACT/Pool, qkt/cp casts split DVE/Pool.
"""
import os
import sys

for _p in ("/opt/trn_rl_repo", "/root/.axon_site/_ro/trn_rl_repo"):
    if os.path.isdir(_p) and _p not in sys.path:
        sys.path.insert(0, _p)

import numpy as np
import ml_dtypes

from contextlib import ExitStack

import concourse.bass as bass
import concourse.tile as tile
from concourse import bacc, mybir
from concourse import bass_utils
from concourse.masks import make_identity

F32 = mybir.dt.float32
BF16 = mybir.dt.bfloat16
FP8 = mybir.dt.float8e4
AF = mybir.ActivationFunctionType
ALU = mybir.AluOpType
PM = mybir.MatmulPerfMode

B, T, C = 2, 2048, 768
H, D = 12, 64
NCORES = 8
GROUPS = [[0, 1, 2, 3], [4, 5, 6, 7]]
HPC = 3            # heads per core
TS = T // 4        # 512: token slice per core (post-RS)
FF = 4 * C         # 3072
NT = T // 128      # 16 token chunks
NCC = C // 128     # 6 channel chunks
NPR = NCC // 2     # 3 contraction pairs (DoubleRow)
NQB = 4            # q blocks
QB = 512
NFC = FF // 128    # 24 hidden chunks
EPS = 1e-5
QKW = 512   # padded qk weight cols: [Q0 Q1 | K0 K1 | Q2 pad | K2 pad]
SQ = 32.0   # fp8 weight scale for wqk/wv/wcp/wfc
SM = 64.0   # fp8 weight scale for wmp
EXP_SCALE = 1.0 / (8.0 * SQ * SQ)   # 1/sqrt(64) / (q,k both carry SQ)

_BUILT = {}


def _pair(t, p, c0, n):
    """[128, 2, n] AP: contraction pair p of a [128, NCC, T]-style tile,
    free cols [c0, c0+n)."""
    return t[:, 2 * p:2 * p + 2, c0:c0 + n]


class _Pools:
    def __init__(self, ctx, tc):
        e = ctx.enter_context
        self.cons = e(tc.tile_pool(name="cons", bufs=1))
        self.xpool = e(tc.tile_pool(name="xpool", bufs=2))
        self.lnpool = e(tc.tile_pool(name="lnpool", bufs=2))
        self.stpool = e(tc.tile_pool(name="stpool", bufs=6))
        self.big2k = e(tc.tile_pool(name="big2k", bufs=1))
        self.qktp = e(tc.tile_pool(name="qktp", bufs=1))
        self.vpool = e(tc.tile_pool(name="vpool", bufs=1))
        self.ptpool = e(tc.tile_pool(name="ptpool", bufs=6))
        self.ytp = e(tc.tile_pool(name="ytp", bufs=1))
        self.invp = e(tc.tile_pool(name="invp", bufs=2))
        self.cpp = e(tc.tile_pool(name="cpp", bufs=4))
        self.rsp = e(tc.tile_pool(name="rsp", bufs=2))
        self.h1p = e(tc.tile_pool(name="h1p", bufs=1))
        self.h2tp = e(tc.tile_pool(name="h2tp", bufs=1))
        self.wfcp = e(tc.tile_pool(name="wfcp", bufs=6))
        self.wmpp = e(tc.tile_pool(name="wmpp", bufs=1))
        self.outp = e(tc.tile_pool(name="outp", bufs=2))
        self.ps = e(tc.tile_pool(name="ps", bufs=4, space="PSUM"))
        self.psyt = e(tc.tile_pool(name="psyt", bufs=3, space="PSUM"))
        self.pstp = e(tc.tile_pool(name="pstp", bufs=1, space="PSUM"))
        self.dram = e(tc.tile_pool(name="dram", bufs=1, space="DRAM"))


def _body(pools, nc, tc, io, timing=False, skip_att=False, skip_mlp=False):
    x, xs, wqk, bqk, wv, wcp, wfc, bfc, wmp, mask, out = io
    cons, xpool, lnpool, stpool = pools.cons, pools.xpool, pools.lnpool, pools.stpool
    big2k, qktp, vpool, ptpool = pools.big2k, pools.qktp, pools.vpool, pools.ptpool
    ytp, invp, cpp, rsp = pools.ytp, pools.invp, pools.cpp, pools.rsp
    h1p, h2tp, wfcp, wmpp = pools.h1p, pools.h2tp, pools.wfcp, pools.wmpp
    outp, ps, psyt, pstp = pools.outp, pools.ps, pools.psyt, pools.pstp
    dram = pools.dram

    # ---- constants ----
    ident = cons.tile([128, 128], BF16)
    make_identity(nc, ident)
    eps_sb = cons.tile([128, 1], F32)
    nc.vector.memset(eps_sb, EPS)
    # -BIG * identity: with the strictly-upper-ones `mask` input this writes
    # a -BIG lower triangle into PSUM via a single 128-cycle PE matmul
    negident = cons.tile([128, 128], BF16)
    nc.vector.tensor_scalar(out=negident, in0=ident, scalar1=-1e6,
                            scalar2=None, op0=ALU.mult)

    # ---- x loads first (batched 2 chunks per DMA), LN1, transpose ----
    hT_big = big2k.tile([128, NCC, T], FP8, name="hT_big", tag="hg")

    def layernorm_chunk(x_t, ln_t):
        # stats on DVE; rstd = (var+eps)^-0.5 in one Pool pow op (no ACT
        # table thrash, no reciprocal); apply on Pool (SBUF-only engine)
        stats = stpool.tile([128, 3, 6], F32)
        mv = stpool.tile([128, 2], F32)
        rstd = stpool.tile([128, 1], F32)
        xg = x_t.rearrange("p (n s) -> p n s", s=256)
        for sg in range(3):
            nc.vector.bn_stats(out=stats[:, sg, :], in_=xg[:, sg, :])
        nc.vector.bn_aggr(out=mv, in_=stats)
        sd = stpool.tile([128, 1], F32)
        nc.scalar.activation(out=sd, in_=mv[:, 1:2], func=AF.Sqrt, bias=eps_sb)
        nc.vector.reciprocal(out=rstd, in_=sd)
        nc.gpsimd.tensor_scalar(out=ln_t, in0=x_t, scalar1=mv[:, 0:1],
                                scalar2=rstd, op0=ALU.subtract, op1=ALU.mult)

    def transpose_chunk(ln_t, dst_big, i, eng):
        # 6 PE transposes into one PSUM row, one batched cast-copy out (fp8)
        tpr = pstp.tile([128, NCC, 128], BF16, name="tpr", tag="tp")
        for j in range(NCC):
            nc.tensor.transpose(out=tpr[:, j, :], in_=ln_t[:, 128 * j:128 * (j + 1)],
                                identity=ident)
        dst = dst_big[:, :, 128 * i:128 * (i + 1)]
        if eng == "act":
            nc.scalar.copy(out=dst, in_=tpr)
        else:
            nc.vector.tensor_copy(out=dst, in_=tpr)

    def x_ln_q(q):
        """DMA one 256-token slab of x, LN + transpose its 2 chunks."""
        xq = xpool.tile([128, 2, C], BF16, name="xq", tag="xq")
        src = x[256 * q:256 * (q + 1), :].rearrange("(r p) c -> p r c", p=128)
        d = nc.sync.dma_start(out=xq, in_=src)
        for r in range(2):
            i = 2 * q + r
            ln_t = lnpool.tile([128, C], BF16, name="ln_t", tag="ln_t")
            layernorm_chunk(xq[:, r, :], ln_t)
            transpose_chunk(ln_t, hT_big, i, eng="act")
        return d

    # late slabs: stats (incl. the ACT sqrt) run before attention so the
    # sqrt<->exp act-table never thrashes; the apply+transpose is deferred
    mv_all = cons.tile([128, 16, 2], F32)
    rstd_all = cons.tile([128, 16], F32)

    def x_stats_q(q):
        xq = xpool.tile([128, 2, C], BF16, name=f"xh{q}", tag=f"xh{q}")
        src = x[256 * q:256 * (q + 1), :].rearrange("(r p) c -> p r c", p=128)
        d = nc.sync.dma_start(out=xq, in_=src)
        for r in range(2):
            i = 2 * q + r
            stats = stpool.tile([128, 3, 6], F32)
            sd = stpool.tile([128, 1], F32)
            xg = xq[:, r, :].rearrange("p (n s) -> p n s", s=256)
            for sg in range(3):
                nc.vector.bn_stats(out=stats[:, sg, :], in_=xg[:, sg, :])
            nc.vector.bn_aggr(out=mv_all[:, i, :], in_=stats)
            nc.scalar.activation(out=sd, in_=mv_all[:, i, 1:2], func=AF.Sqrt,
                                 bias=eps_sb)
            nc.vector.reciprocal(out=rstd_all[:, i:i + 1], in_=sd)
        return xq, d

    def x_apply_q(xq, q):
        for r in range(2):
            i = 2 * q + r
            ln_t = lnpool.tile([128, C], BF16, name="ln_t", tag="ln_t")
            nc.gpsimd.tensor_scalar(out=ln_t, in0=xq[:, r, :],
                                    scalar1=mv_all[:, i, 0:1],
                                    scalar2=rstd_all[:, i:i + 1],
                                    op0=ALU.subtract, op1=ALU.mult)
            transpose_chunk(ln_t, hT_big, i, eng="act")

    # ---- x slabs 0-1 first so LN starts immediately; then the small
    # weights (needed by qk(0) at ~6us); then slabs 2-3. Remaining slabs are
    # emitted inside the attention loop ahead of the qk block needing them.
    x_ln_q(0)
    x_ln_q(1)

    mask_sb = cons.tile([128, 128], BF16)
    nc.sync.dma_start(out=mask_sb, in_=mask)

    wqk_sb = []
    wv_sb = []
    for p in range(NPR):
        wq_t = cons.tile([128, 2, QKW], FP8, name=f"wqk{p}", tag=f"wqk{p}")
        nc.sync.dma_start(out=wq_t, in_=wqk[p])
        wqk_sb.append(wq_t)
        wv_t = cons.tile([128, 2, HPC * D], FP8, name=f"wv{p}", tag=f"wv{p}")
        nc.sync.dma_start(out=wv_t, in_=wv[p])
        wv_sb.append(wv_t)
    wcp_sb = cons.tile([128, 2, C], FP8, name="wcp", tag="wcp")
    nc.sync.dma_start(out=wcp_sb, in_=wcp)

    def _col_bias(name, src, n):
        t = cons.tile([128, n], F32, name=name, tag=name)
        nc.sync.dma_start(out=t, in_=src.rearrange("(g p) -> p g", p=128))
        return t

    bqk_sb = _col_bias("bqk_sb", bqk, QKW // 128)   # [128, 4] (pre-scaled by SQ)
    bfc_sb = _col_bias("bfc_sb", bfc, NFC)          # [128, 24] (true scale)

    x_ln_q(2)
    x_ln_q(3)
    xh4, _ = x_stats_q(4)
    xh5, _ = x_stats_q(5)
    xh6, _ = x_stats_q(6)
    xh7, x_dma_last = x_stats_q(7)
    xh = {4: xh4, 5: xh5, 6: xh6, 7: xh7}

    # ---- phase 3: QK^T [512, T] (padded layout) and V_aug [t, 3, 65] ----
    # emission is interleaved with the attention blocks (see below) so the
    # first attention block starts as soon as x chunks 0-3 are through.
    qkT = [qktp.tile([128, T], BF16, name=f"qkt{g}", tag=f"qkt{g}") for g in range(4)]

    def qk_block(g, n):
        acc = ps.tile([128, QB], F32, name="acc", tag="acc")
        for p in range(NPR):
            nc.tensor.matmul(out=acc,
                             lhsT=wqk_sb[p][:, :, 128 * g:128 * (g + 1)],
                             rhs=_pair(hT_big, p, QB * n, QB),
                             start=(p == 0), stop=(p == NPR - 1),
                             perf_mode=PM.DoubleRow)
        dst = qkT[g][:, QB * n:QB * (n + 1)]
        if n < 3:
            # early q-blocks: ACT is still idle (exp hasn't started)
            nc.scalar.activation(out=dst, in_=acc, func=AF.Identity,
                                 bias=bqk_sb[:, g:g + 1])
        else:
            nc.vector.tensor_scalar_add(out=dst, in0=acc,
                                        scalar1=bqk_sb[:, g:g + 1])

    v_sb = []

    def v_chunk(i):
        v_t = vpool.tile([128, HPC, D + 1], BF16, name=f"v{i}", tag=f"v{i}")
        nc.vector.memset(v_t[:, :, D:D + 1], 1.0)
        acc = ps.tile([128, QB], F32, name="acc", tag="acc")
        for p in range(NPR):
            nc.tensor.matmul(out=acc[:, :HPC * D],
                             lhsT=_pair(hT_big, p, 128 * i, 128),
                             rhs=wv_sb[p],
                             start=(p == 0), stop=(p == NPR - 1),
                             perf_mode=PM.DoubleRow)
        nc.vector.tensor_copy(
            out=v_t[:, :, 0:D],
            in_=acc[:, :HPC * D].rearrange("p (h d) -> p h d", d=D))
        v_sb.append(v_t)

    # head h: Q^T in group [0,0,2][h] at partition offset [0,64,0][h];
    # K^T in the following group at the SAME offset (matmul quadrant rule).
    def qT_slice(h, nq):
        g, off = (0 if h < 2 else 2), 64 * (h % 2)
        return qkT[g][off:off + 64, QB * nq:QB * (nq + 1)]

    def kT_slice(h, kc):
        g, off = (1 if h < 2 else 3), 64 * (h % 2)
        return qkT[g][off:off + 64, 128 * kc:128 * (kc + 1)]

    # ---- phase 4: attention ----
    # yT layout for DoubleRow c_proj: [128, 2, T] fp8, scale SQ:
    #   ktile0 = heads 0,1 (d0-63 at part 0-63 / 64-127); ktile1 = head 2 +
    #   zero pad at part 64-127.
    yT = ytp.tile([128, 2, T], FP8, name="yT", tag="yT")
    nc.gpsimd.memset(yT[64:128, 1, :], 0.0)

    def yT_slice(h, nq):
        off, kt = [(0, 0), (64, 0), (0, 1)][h]
        return yT[off:off + 64, kt, QB * nq:QB * (nq + 1)]

    # ---- phases 4+5 interleaved: attention q-blocks, c_proj per block,
    # RS-A fired after q-block 1, LN2-A during the attention tail ----
    rs_inA = dram.tile([T // 2, C], BF16)
    rs_inB = dram.tile([T // 2, C], BF16)
    rs_outA = dram.tile([TS // 2, C], BF16)
    rs_outB = dram.tile([TS // 2, C], BF16)
    cp_scale = cons.tile([128, 1], F32)
    nc.vector.memset(cp_scale, 1.0 / (SQ * SQ))
    h1 = [h1p.tile([128, C], F32, name=f"h1_{i}", tag=f"h1_{i}") for i in range(4)]
    h2T_big = h2tp.tile([128, NCC, TS], FP8, name="h2T_big", tag="h2t")
    h2T = h2T_big

    def attention_block(nq):
        nk = 4 * (nq + 1)
        for h in range(HPC):
            yt = psyt.tile([D + 1, QB], F32, name="yt", tag="yt")
            for kc in range(nk):
                j = kc - 4 * nq
                f0 = max(0, 128 * j)   # cols < f0 are fully masked
                st = ps.tile([128, QB], F32, name="st", tag="acc")
                if j >= 0:
                    # diagonal chunk: full-width scores first (start=True
                    # resets the whole PSUM bank zero-region), then add a
                    # -BIG triangle on the 128 diagonal cols via a 128-cycle
                    # PE matmul (mask_sb = strictly-upper ones) -> exp gives
                    # exact zeros, no mask multiply needed
                    nc.tensor.matmul(out=st[:, f0:], lhsT=kT_slice(h, kc),
                                     rhs=qT_slice(h, nq)[:, f0:],
                                     start=True, stop=False)
                    nc.tensor.matmul(out=st[:, f0:f0 + 128], lhsT=mask_sb,
                                     rhs=negident, start=False, stop=True)
                else:
                    nc.tensor.matmul(out=st[:, f0:], lhsT=kT_slice(h, kc),
                                     rhs=qT_slice(h, nq)[:, f0:],
                                     start=True, stop=True)
                pt = ptpool.tile([128, QB], BF16, name="pt", tag="pt")
                nc.scalar.activation(out=pt[:, f0:], in_=st[:, f0:],
                                     func=AF.Exp, scale=EXP_SCALE)
                nc.tensor.matmul(out=yt[:, f0:], lhsT=v_sb[kc][:, h, :],
                                 rhs=pt[:, f0:],
                                 start=(kc == 0), stop=(kc == nk - 1))
            inv = invp.tile([1, QB], F32, name="inv", tag="inv")
            nc.vector.reciprocal(out=inv, in_=yt[D:D + 1, :])
            invb = invp.tile([64, QB], F32, name="invb", tag="invb")
            nc.gpsimd.partition_broadcast(invb, inv)
            nc.vector.tensor_tensor(out=yT_slice(h, nq),
                                    in0=yt[0:D, :], in1=invb, op=ALU.mult)

    def cproj_chunk(i):
        cp_t = cpp.tile([128, C], BF16, name="cp_t", tag="cp_t")
        for fr in range(2):
            acc = ps.tile([128, 384], F32, name="acc2", tag="acc")
            nc.tensor.matmul(out=acc, lhsT=yT[:, :, 128 * i:128 * (i + 1)],
                             rhs=wcp_sb[:, :, 384 * fr:384 * (fr + 1)],
                             start=True, stop=True, perf_mode=PM.DoubleRow)
            dst = cp_t[:, 384 * fr:384 * (fr + 1)]
            if fr == 0:
                nc.scalar.mul(out=dst, in_=acc, mul=1.0 / (SQ * SQ))
            else:
                nc.vector.tensor_scalar(out=dst, in0=acc, scalar1=cp_scale,
                                        scalar2=None, op0=ALU.mult)
        if i < NT // 2:
            nc.sync.dma_start(out=rs_inA[128 * i:128 * (i + 1), :], in_=cp_t)
        else:
            ii = i - NT // 2
            nc.sync.dma_start(out=rs_inB[128 * ii:128 * (ii + 1), :], in_=cp_t)

    def rs_fire(half):
        rs_in, rs_out = (rs_inA, rs_outA) if half == 0 else (rs_inB, rs_outB)
        if timing:
            # timing build (TimelineSim/loop can't model collectives): stand-in
            nc.sync.dma_start(out=rs_out, in_=rs_in[0:TS // 2, :])
        else:
            nc.gpsimd.collective_compute(
                "ReduceScatter", ALU.add, replica_groups=GROUPS,
                ins=[rs_in.opt()], outs=[rs_out.opt()])

    def post_rs(q):
        # residual + LN2 + transpose for the 256 tokens of RS half q
        rs_q = rsp.tile([128, 2, C], BF16, name="rs_q", tag="rs_q")
        rs_src = rs_outA if q == 0 else rs_outB
        nc.sync.dma_start(
            out=rs_q,
            in_=rs_src.rearrange("(r p) c -> p r c", p=128))
        xs_q = xpool.tile([128, 2, C], F32, name="xsq", tag="xsq")
        nc.sync.dma_start(
            out=xs_q,
            in_=xs[256 * q:256 * (q + 1), :].rearrange("(r p) c -> p r c", p=128))
        for r in range(2):
            i = 2 * q + r
            nc.gpsimd.tensor_tensor(out=h1[i], in0=xs_q[:, r, :], in1=rs_q[:, r, :],
                                    op=ALU.add)
            ln_t = lnpool.tile([128, C], BF16, name="ln_t", tag="ln_t")
            layernorm_chunk(h1[i], ln_t)
            transpose_chunk(ln_t, h2T_big, i, eng="vec")

    if skip_att:
        x_ln_q(4)
        x_ln_q(5)
        x_ln_q(6)
        x_dma_last = x_ln_q(7)
        for n in range(NQB):
            for g in range(4):
                qk_block(g, n)
        for i in range(NT):
            v_chunk(i)
        for h in range(HPC):
            for nq in range(NQB):
                nc.vector.memset(yT_slice(h, nq), 0.001)
        for i in range(NT):
            cproj_chunk(i)
        rs_fire(0)
        rs_fire(1)
        post_rs(0)
        post_rs(1)
    else:
        # emit qk/v one q-block ahead of the attention consuming it, so the
        # casts feeding exp(nq+1) are never gated by attention(nq)'s PE tail
        for g in range(4):
            qk_block(g, 0)
        for i in range(4):
            v_chunk(i)
        for nq in range(NQB):
            attention_block(nq)
            if nq + 1 < NQB:
                # exp(nq+1)'s feeders first: qk casts, then v
                for g in range(4):
                    qk_block(g, nq + 1)
                for i in range(4 * nq + 4, 4 * nq + 8):
                    v_chunk(i)
            for i in range(4 * nq, 4 * nq + 4):
                cproj_chunk(i)
            if nq == 1:
                rs_fire(0)
            if nq <= 1:
                # LN apply for the x slabs feeding qk(nq+2)
                x_apply_q(xh[2 * nq + 4], 2 * nq + 4)
                x_apply_q(xh[2 * nq + 5], 2 * nq + 5)
        rs_fire(1)
        post_rs(0)
        post_rs(1)

    # ---- phase 8: MLP ----
    if skip_mlp:
        for i in range(4):
            out_t = outp.tile([128, C], F32, name="out_t", tag="out_t")
            nc.vector.tensor_copy(out=out_t, in_=h1[i])
            nc.sync.dma_start(out=out[128 * i:128 * (i + 1), :], in_=out_t)
        return
    # fc weights streamed as [128, 2, 768] fp8 slabs: 12 DMAs
    gl_big = big2k.tile([128, NCC, T], BF16, name="gl_big", tag="hg")
    wmp_sb = [None] * (FF // 256)

    def wmp_load(p):
        wmp_t = wmpp.tile([128, 2, C], BF16, name=f"wmp{p}", tag=f"wmp{p}")
        d = nc.sync.dma_start(out=wmp_t, in_=wmp[p])
        tile.add_dep_helper(d.ins, x_dma_last.ins, sync=False,
                            reason="defer wmp prefetch past x load")
        wmp_sb[p] = wmp_t

    def gl_chunk(fi, i):
        # [128, 128] bf16: hidden chunk fi at token cols 128i
        jj, m = fi // 4, fi % 4
        return gl_big[:, jj, TS * m + 128 * i:TS * m + 128 * (i + 1)]

    # mproj groups (i, cr) for i<2 are split: the fi<12 half-contraction runs
    # during the fc/gelu phase (PE is gelu-gated there) in the attention-era
    # PSUM banks; the fi>=12 half + combine run after fc
    accA_list = []
    for fg in range(4):
        slabs = []
        for p in range(NPR):
            wfc_t = wfcp.tile([128, 2, 768], FP8, name="wfc_t", tag="wfc_t")
            d = nc.sync.dma_start(
                out=wfc_t, in_=wfc[p][:, :, 768 * fg:768 * (fg + 1)])
            tile.add_dep_helper(d.ins, x_dma_last.ins, sync=False,
                                reason="defer wfc prefetch past x load")
            slabs.append(wfc_t)
        for fl in range(6):
            fi = 6 * fg + fl
            acc = ps.tile([128, TS], F32, name="accf", tag="acc")
            for p in range(NPR):
                nc.tensor.matmul(out=acc,
                                 lhsT=slabs[p][:, :, 128 * fl:128 * (fl + 1)],
                                 rhs=_pair(h2T, p, 0, TS),
                                 start=(p == 0), stop=(p == NPR - 1),
                                 perf_mode=PM.DoubleRow)
            jj, m = fi // 4, fi % 4
            nc.scalar.activation(out=gl_big[:, jj, TS * m:TS * (m + 1)], in_=acc,
                                 func=AF.Gelu, scale=1.0 / SQ,
                                 bias=bfc_sb[:, fi:fi + 1])
        if fg == 1:
            for p in range(6):
                wmp_load(p)
            for g in range(4):
                ii, cr = g // 2, g % 2
                if g < 3:
                    accA = psyt.tile([128, 384], F32, name=f"accA{g}", tag="yt")
                else:
                    accA = pstp.tile([128, 384], F32, name="accA3", tag="tp")
                accA_list.append(accA)
                for fi in range(12):
                    p, kt = fi // 2, fi % 2
                    nc.tensor.matmul(out=accA, lhsT=gl_chunk(fi, ii),
                                     rhs=wmp_sb[p][:, kt, 384 * cr:384 * (cr + 1)],
                                     start=(fi == 0), stop=(fi == 11))
        if fg == 3:
            for p in range(6, 12):
                wmp_load(p)


    for i in range(4):
        out_t = outp.tile([128, C], F32, name="out_t", tag="out_t")
        for cr in range(2):
            sl = slice(384 * cr, 384 * (cr + 1))
            if i < 2:
                g = 2 * i + cr
                accB = ps.tile([128, 384], F32, name="accm", tag="acc")
                for fi in range(12, NFC):
                    p, kt = fi // 2, fi % 2
                    nc.tensor.matmul(out=accB, lhsT=gl_chunk(fi, i),
                                     rhs=wmp_sb[p][:, kt, 384 * cr:384 * (cr + 1)],
                                     start=(fi == 12), stop=(fi == NFC - 1))
                tmp = cpp.tile([128, 384], F32, name="mtmp", tag="mtmp")
                nc.vector.tensor_tensor(out=tmp, in0=accA_list[g],
                                        in1=h1[i][:, sl], op=ALU.add)
                nc.vector.tensor_tensor(out=out_t[:, sl], in0=accB,
                                        in1=tmp, op=ALU.add)
            else:
                acc = ps.tile([128, 384], F32, name="accm", tag="acc")
                for fi in range(NFC):
                    p, kt = fi // 2, fi % 2
                    nc.tensor.matmul(out=acc, lhsT=gl_chunk(fi, i),
                                     rhs=wmp_sb[p][:, kt, 384 * cr:384 * (cr + 1)],
                                     start=(fi == 0), stop=(fi == NFC - 1))
                nc.vector.tensor_tensor(out=out_t[:, sl], in0=acc,
                                        in1=h1[i][:, sl], op=ALU.add)
        nc.sync.dma_start(out=out[128 * i:128 * (i + 1), :], in_=out_t)


def build(timing=False, loop_n=1, skip_att=False, skip_mlp=False):
    key = ("nc", timing, loop_n, skip_att, skip_mlp)
    if key in _BUILT:
        return _BUILT[key]
    nc = bacc.Bacc("TRN2", target_bir_lowering=False, debug=False,
                   num_devices=1 if timing else NCORES)

    def din(name, shape, dt):
        return nc.dram_tensor(name, shape, dt, kind="ExternalInput").ap()

    io = (
        din("x", [T, C], BF16),
        din("xs", [TS, C], F32),
        din("wqk", [NPR, 128, 2, QKW], FP8),
        din("bqk", [QKW], F32),
        din("wv", [NPR, 128, 2, HPC * D], FP8),
        din("wcp", [128, 2, C], FP8),
        din("wfc", [NPR, 128, 2, FF], FP8),
        din("bfc", [FF], F32),
        din("wmp", [FF // 256, 128, 2, C], BF16),
        din("mask", [128, 128], BF16),
        nc.dram_tensor("out", [TS, C], F32, kind="ExternalOutput").ap(),
    )
    with tile.TileContext(nc) as tc, ExitStack() as ctx:
        pools = _Pools(ctx, tc)
        if loop_n > 1:
            with tc.For_i(0, loop_n, 1):
                _body(pools, nc, tc, io, timing=True,
                      skip_att=skip_att, skip_mlp=skip_mlp)
        else:
            _body(pools, nc, tc, io, timing=timing,
                  skip_att=skip_att, skip_mlp=skip_mlp)
    nc.finalize()
    _BUILT[key] = nc
    return nc


def make_in_maps(inputs):
    """Host-side sharding: full inputs dict -> per-core in_maps."""
    f32 = np.float32
    bf = ml_dtypes.bfloat16
    f8 = ml_dtypes.float8_e4m3
    x = np.asarray(inputs["x"], f32)
    ln1_g = np.asarray(inputs["ln1_g"], f32)
    ln1_b = np.asarray(inputs["ln1_b"], f32)
    W_attn = np.asarray(inputs["W_attn"], f32)
    b_attn = np.asarray(inputs["b_attn"], f32)
    W_cproj = np.asarray(inputs["W_cproj"], f32)
    b_cproj = np.asarray(inputs["b_cproj"], f32)
    ln2_g = np.asarray(inputs["ln2_g"], f32)
    ln2_b = np.asarray(inputs["ln2_b"], f32)
    W_fc = np.asarray(inputs["W_fc"], f32)
    b_fc = np.asarray(inputs["b_fc"], f32)
    W_mproj = np.asarray(inputs["W_mproj"], f32)
    b_mproj = np.asarray(inputs["b_mproj"], f32)

    Wa = ln1_g[:, None] * W_attn
    ba = b_attn + ln1_b @ W_attn
    Wf = ln2_g[:, None] * W_fc
    bf_ = b_fc + ln2_b @ W_fc

    assert not np.any(ba[1536:]), "nonzero v bias unsupported in fp8 kernel"
    assert not np.any(b_mproj), "nonzero mproj bias unsupported in fp8 kernel"

    def dr_pairs(w, dt=None):
        """[256k-rows, M] -> [128, 2, M] tile content per row-pair."""
        return np.ascontiguousarray(
            w.reshape(2, 128, -1).transpose(1, 0, 2).astype(dt or f8))

    p = np.arange(128)[:, None]
    c = np.arange(128)[None, :]
    # lhsT for the PE triangle trick: mask[d, p] = 1 iff d < p
    mask = (p < c).astype(bf)

    wcp_all = (W_cproj * SQ).astype(f32)
    wfc_all = (Wf * SQ).astype(f32)
    wfc_dr = np.stack([dr_pairs(wfc_all[256 * p0:256 * (p0 + 1)])
                       for p0 in range(NPR)])
    wmp_dr = np.stack([dr_pairs(W_mproj[256 * p0:256 * (p0 + 1)], bf)
                       for p0 in range(FF // 256)])

    maps = []
    for core in range(NCORES):
        b, s = core // 4, core % 4
        q0 = 192 * s
        zpad = np.zeros((C, 64), f32)
        # [Q0 Q1 | K0 K1 | Q2 pad | K2 pad], scaled by SQ
        wqk_full = np.concatenate([
            Wa[:, q0:q0 + 128], Wa[:, 768 + q0:768 + q0 + 128],
            Wa[:, q0 + 128:q0 + 192], zpad,
            Wa[:, 768 + q0 + 128:768 + q0 + 192], zpad], axis=1) * SQ
        bqk = np.concatenate([
            ba[q0:q0 + 128], ba[768 + q0:768 + q0 + 128],
            ba[q0 + 128:q0 + 192], np.zeros(64, f32),
            ba[768 + q0 + 128:768 + q0 + 192], np.zeros(64, f32)]) * SQ
        wv_full = Wa[:, 1536 + q0:1536 + q0 + 192] * SQ
        # c_proj rows for this core's heads: ktile0 = rows 0-127 (heads 0,1),
        # ktile1 = rows 128-191 (head 2) + 64 zero rows
        wcp_rows = np.concatenate([wcp_all[q0:q0 + 192, :],
                                   np.zeros((64, C), f32)], axis=0)
        maps.append({
            "x": np.ascontiguousarray(x[b]).astype(bf),
            "xs": np.ascontiguousarray(np.concatenate([
                x[b, 256 * s:256 * s + 256],
                x[b, 1024 + 256 * s:1024 + 256 * s + 256]])
                + b_cproj),
            "wqk": np.stack([dr_pairs(wqk_full[256 * p0:256 * (p0 + 1)])
                             for p0 in range(NPR)]),
            "bqk": np.ascontiguousarray(bqk),
            "wv": np.stack([dr_pairs(wv_full[256 * p0:256 * (p0 + 1)])
                            for p0 in range(NPR)]),
            "wcp": dr_pairs(wcp_rows),
            "wfc": wfc_dr,
            "bfc": bf_,
            "wmp": wmp_dr,
            "mask": mask,
        })
    return maps


def _get_runner():
    """Persistent jitted 8-core dispatch (replicates bass2jax.run_bass_via_pjrt
    but keeps the compiled executable so repeated kernel() calls are cheap)."""
    if "runner" in _BUILT:
        return _BUILT["runner"]
    import jax
    from jax.sharding import Mesh, PartitionSpec, NamedSharding
    from jax.experimental.shard_map import shard_map
    from concourse import bass2jax

    nc = build()
    bass2jax.install_neuronx_cc_hook()
    part_name = nc.partition_id_tensor.name if nc.partition_id_tensor else None
    in_names, out_names, out_avals, zero_shapes = [], [], [], []
    for alloc in nc.m.functions[0].allocations:
        if not isinstance(alloc, mybir.MemoryLocationSet):
            continue
        name = alloc.memorylocations[0].name
        if alloc.kind == "ExternalInput":
            if name != part_name:
                in_names.append(name)
        elif alloc.kind == "ExternalOutput":
            out_names.append(name)
            shape = tuple(alloc.tensor_shape)
            dtype = mybir.dt.np(alloc.dtype)
            out_avals.append(jax.core.ShapedArray(shape, dtype))
            zero_shapes.append((shape, dtype))
    n_params, n_outs = len(in_names), len(out_names)
    all_names = in_names + out_names + ([part_name] if part_name else [])

    def _fn(*args):
        args = list(args)
        if part_name is not None:
            args.append(bass2jax.partition_id_tensor())
        return tuple(bass2jax.bass_exec(out_avals, all_names, out_names, nc, {},
                                        True, True, *args))

    devices = jax.devices()[:NCORES]
    mesh = Mesh(np.asarray(devices), ("core",))
    sharded = jax.jit(
        shard_map(_fn, mesh=mesh,
                  in_specs=(PartitionSpec("core"),) * (n_params + n_outs),
                  out_specs=(PartitionSpec("core"),) * n_outs, check_rep=False),
        donate_argnums=tuple(range(n_params, n_params + n_outs)), keep_unused=True)
    sh = NamedSharding(mesh, PartitionSpec("core"))

    def run(maps):
        concat_in = [jax.device_put(np.concatenate(
            [np.asarray(maps[c][nm]) for c in range(NCORES)], axis=0), sh)
            for nm in in_names]
        zeros = [jax.device_put(
            np.zeros((NCORES * shp[0], *shp[1:]), dt), sh)
            for shp, dt in zero_shapes]
        outs = sharded(*concat_in, *zeros)
        i = out_names.index("out")
        return np.asarray(outs[i]).reshape(NCORES, TS, C)

    _BUILT["runner"] = run
    return run


def kernel(**inputs):
    maps = make_in_maps(inputs)
    run = _get_runner()
    per_core = run(maps)
    out = np.empty((B, T, C), np.float32)
    for core in range(NCORES):
        b, s = core // 4, core % 4
        out[b, 256 * s:256 * s + 256] = per_core[core][0:256]
        out[b, 1024 + 256 * s:1024 + 256 * s + 256] = per_core[core][256:512]
    return out
